# revision 29
# baseline (speedup 1.0000x reference)
"""Trainium2 Bass kernel for a hidden-size-1 GRU over M=65536 independent
sequences (T=12 steps, FE=32 features), followed by relu + linear head.

Strategy (data-parallel over 8 NeuronCores, 8192 sequences each):
  - x is cast to bf16 on the host and shipped as [3 chunks, 2 halves,
    128, 4096] so each DMA is one contiguous 1 MiB read; this halves HBM
    traffic (6.3 MiB/core) and the matmul runs at bf16 rate.
  - Gate projection with x as the STATIONARY operand: per 128-sequence
    block, lhsT = x chunk [128 rows = 4 timesteps x 32 features, 128 seqs],
    rhs = a block-diagonal bf16 weight [128, 12], so gates land directly
    as [128 seqs, 12 (t,gate)] tiles in PSUM -- no PE transposes and no
    narrow PSUM copies (the two dominant costs of the previous version).
  - Gate biases are folded into the recurrence's ScalarE activation
    immediates, so the PSUM->SBUF copy is a plain dense [128, 384] copy.
  - The GRU recurrence runs batched in 2 streams of 32 blocks (aligned
    with DMA halves) to hide cross-engine latency; per step it uses the
    form h' = n + z*(h-n) with ops split across Scalar/Vector/GpSimd.
  - relu + linear head: broadcast-multiply with a precomputed lin_w tile
    and a strided tensor_reduce over T, split across Vector and GpSimd.
"""

import numpy as np

B, N, FE, T, OUT = 32, 2048, 32, 12, 3
M = B * N
NCORES = 8
MC = M // NCORES          # 8192 sequences per core
C = FE * T                # 384 contraction length
NBLK = MC // 128          # 64 column blocks of 128 sequences
NS = 2                    # recurrence streams (= DMA halves per chunk)
SB = NBLK // NS           # 32 blocks per stream
HW = MC // NS // 128 * 128 * NS // NS // 1  # unused; kept simple below

_COMPILED = None          # (nc, weights_key)


def _build_program(w_hh, b_hh, b_ih, lin_b):
    """Build the bass program. Recurrent weights/biases are baked as
    immediates (the kernel is JIT-compiled per weight values)."""
    from contextlib import ExitStack

    import concourse.bass as bass
    import concourse.tile as tile
    from concourse import mybir

    f32 = mybir.dt.float32
    bf16 = mybir.dt.bfloat16
    AF = mybir.ActivationFunctionType
    add, mult, sub = (
        mybir.AluOpType.add,
        mybir.AluOpType.mult,
        mybir.AluOpType.subtract,
    )
    wh0, wh1, wh2 = (float(w_hh[i]) for i in range(3))
    br = float(b_ih[0] + b_hh[0])
    bz = float(b_ih[1] + b_hh[1])
    bn = float(b_ih[2])
    bhh2 = float(b_hh[2])

    nc = bass.Bass("TRN2", target_bir_lowering=False, debug=False)

    xd = nc.dram_tensor("xt", [3, NS, 128, 8192 // NS], bf16, kind="ExternalInput").ap()
    w3d = nc.dram_tensor("w3", [3, 128, 12], bf16, kind="ExternalInput").ap()
    lwd = nc.dram_tensor("lwb", [128, OUT * T * SB * NS], bf16, kind="ExternalInput").ap()
    outd = nc.dram_tensor("out", [128, OUT, NS * SB], f32, kind="ExternalOutput").ap()

    with ExitStack() as ctx:
        tc = ctx.enter_context(tile.TileContext(nc))
        consts = ctx.enter_context(tc.tile_pool(name="consts", bufs=1))
        xpool = ctx.enter_context(tc.tile_pool(name="x", bufs=1))
        gpool = ctx.enter_context(tc.tile_pool(name="g", bufs=1))
        work = ctx.enter_context(tc.tile_pool(name="work", bufs=2))
        psum_gp = ctx.enter_context(tc.tile_pool(name="pgp", bufs=4, space="PSUM"))

        # --- constants ---
        w3_sb = consts.tile([128, 36], bf16, tag="w3sb")
        # bias row for the PSUM bias matmul: col b*12 + tl*3 + g -> bias_g
        ones_sb = consts.tile([1, 128], bf16, tag="ones")
        nc.vector.memset(ones_sb, 1.0)
        brow_sb = consts.tile([1, SB * 12], bf16, tag="brow")
        for g, val in enumerate([br, bz, bn]):
            nc.vector.memset(brow_sb.rearrange("p (k g) -> p k g", g=3)[:, :, g], val)

        # --- x stream: 6 contiguous 1 MiB DMAs, chunk-major.  The first x
        # DMA goes out before the small w3/lwb constants so chunk 0 lands as
        # early as possible (the queue drains FIFO).
        xs = {}
        for j in range(3):
            for s in range(NS):
                xs[(j, s)] = xpool.tile(
                    [128, 8192 // NS], bf16, tag=f"x{j}_{s}", name=f"x{j}_{s}"
                )
        # chunk 0 goes out in 256 KB quarters so its matmuls overlap the
        # DMA (each quarter covers 8 whole blocks); later chunks arrive
        # during the recurrence and stay as single 1 MiB transfers.
        QW = 8192 // NS // 4
        nc.sync.dma_start(out=xs[(0, 0)][:, :QW], in_=xd[0, 0, :, :QW])
        for j in range(3):
            nc.sync.dma_start(out=w3_sb[:, j * 12 : (j + 1) * 12], in_=w3d[j])
        for q in range(1, 4):
            nc.sync.dma_start(
                out=xs[(0, 0)][:, q * QW : (q + 1) * QW],
                in_=xd[0, 0, :, q * QW : (q + 1) * QW],
            )
        for q in range(4):
            nc.sync.dma_start(
                out=xs[(0, 1)][:, q * QW : (q + 1) * QW],
                in_=xd[0, 1, :, q * QW : (q + 1) * QW],
            )
        for j in range(1, 3):
            for s in range(NS):
                nc.sync.dma_start(out=xs[(j, s)], in_=xd[j, s])

        # lin_w broadcast tile, loaded after x (needed only for the head)
        lw2_sb = consts.tile([128, OUT * T * SB * NS], bf16, tag="lwsb")
        nc.sync.dma_start(out=lw2_sb, in_=lwd)

        # --- gates: per (chunk, stream): 32 x (ldweights + matmul N=12),
        # then one dense [128, 384] PSUM->SBUF copy into G[s] ---
        # G[s] column layout: (t*3+gate)*SB + k  (tg-major, block-minor)
        G = {}
        for s in range(NS):
            G[s] = gpool.tile([128, 36 * SB], f32, tag=f"G{s}", name=f"G{s}")
        for j in range(3):
            for s in range(NS):
                gp = psum_gp.tile([128, SB * 12], f32, tag="gp", name=f"gp{j}_{s}")
                nc.tensor.matmul(
                    gp,
                    lhsT=ones_sb,
                    rhs=brow_sb,
                    start=True,
                    stop=False,
                    skip_group_check=True,
                )
                for b in range(SB):
                    nc.tensor.matmul(
                        gp[:, b * 12 : (b + 1) * 12],
                        lhsT=xs[(j, s)][:, b * 128 : (b + 1) * 128],
                        rhs=w3_sb[:, j * 12 : (j + 1) * 12],
                        start=False,
                        stop=(b == SB - 1),
                        skip_group_check=True,
                    )
                G3 = G[s].rearrange("p (tg k) -> p tg k", tg=36)
                nc.scalar.activation(
                    G3[:, j * 12 : (j + 1) * 12, :],
                    gp.rearrange("p (k tg) -> p tg k", k=SB),
                    AF.Copy,
                    bias=0.0,
                )

        # --- GRU recurrence, 2 streams of [128, 32] tiles ---
        # h' = n + z*(h - n).  Gate biases already live in G (PSUM bias
        # matmul).  Per stream, r and z share one [128, 64] sigmoid.
        # Emission is software-pipelined (independent ops queued before
        # dependent ones) and DVE work is balanced: Vector runs the cheap
        # STT forms, GpSimd runs u/v and stream-0's tail as tensor_tensor.
        # H is stored k-major (col = k*T + t) so the head is contiguous.
        one = 1.0
        H = {}
        Hv = {}
        for s in range(NS):
            H[s] = gpool.tile([128, T * SB], f32, tag=f"H{s}", name=f"H{s}")
            Hv[s] = H[s].rearrange("p (k t) -> p t k", t=T)

        def gcol(s, t, gate):
            return G[s][:, (t * 3 + gate) * SB : (t * 3 + gate + 1) * SB]

        def wtile(tag, t, s, width=SB):
            return work.tile([128, width], f32, tag=tag, name=f"{tag}_{t}_{s}")

        for t in range(T):
            rz, v, w, n, e, ze = ({} for _ in range(6))
            ht = {s: Hv[s][:, t, :] for s in range(NS)}
            hp = {s: Hv[s][:, t - 1, :] for s in range(NS)} if t else None
            for s in range(NS):
                rz[s] = wtile("rz", t, s, 2 * SB)
                for tag, d in (("v", v), ("w", w), ("n", n), ("e", e), ("ze", ze)):
                    d[s] = wtile(tag, t, s)
            if t == 0:
                for s in range(NS):
                    nc.scalar.activation(rz[s], G[s][:, 0 : 2 * SB], AF.Sigmoid)
                for s in range(NS):
                    nc.vector.tensor_scalar_mul(v[s], rz[s][:, :SB], bhh2)
                    nc.vector.scalar_tensor_tensor(
                        w[s], v[s], one, gcol(s, 0, 2), mult, add
                    )
                for s in range(NS):
                    nc.scalar.activation(n[s], w[s], AF.Tanh)
                nc.gpsimd.tensor_scalar_mul(e[0], n[0], -1.0)
                nc.vector.tensor_scalar_mul(e[1], n[1], -1.0)
            else:
                srz = {s: wtile("srz", t, s, 2 * SB) for s in range(NS)}
                u = {s: wtile("u", t, s) for s in range(NS)}
                for s in range(NS):
                    nc.vector.scalar_tensor_tensor(
                        srz[s][:, :SB], hp[s], wh0, gcol(s, t, 0), mult, add
                    )
                    nc.vector.scalar_tensor_tensor(
                        srz[s][:, SB:], hp[s], wh1, gcol(s, t, 1), mult, add
                    )
                for s in range(NS):
                    nc.scalar.activation(rz[s], srz[s], AF.Sigmoid)
                for s in range(NS):
                    nc.gpsimd.tensor_scalar(u[s], hp[s], wh2, bhh2, mult, add)
                for s in range(NS):
                    nc.vector.scalar_tensor_tensor(
                        v[s], rz[s][:, :SB], one, u[s], mult, mult
                    )
                    nc.vector.scalar_tensor_tensor(
                        w[s], v[s], one, gcol(s, t, 2), mult, add
                    )
                for s in range(NS):
                    nc.scalar.activation(n[s], w[s], AF.Tanh)
                nc.gpsimd.tensor_tensor(e[0], hp[0], n[0], sub)
                nc.vector.scalar_tensor_tensor(e[1], hp[1], one, n[1], mult, sub)
            nc.gpsimd.tensor_mul(ze[0], rz[0][:, SB:], e[0])
            nc.gpsimd.tensor_add(ht[0], n[0], ze[0])
            nc.vector.scalar_tensor_tensor(ze[1], rz[1][:, SB:], one, e[1], mult, mult)
            nc.vector.scalar_tensor_tensor(ht[1], n[1], one, ze[1], mult, add)

        # --- relu + linear head + output DMA, merged across streams ---
        # HR_AB holds both streams' relu(H) side by side (col = (s*SB+k)*T+t,
        # k-major within stream), so each output channel is ONE GpSimd
        # multiply + ONE wide Vector reduce.  Output DMAs go per channel so
        # the last (receipt-latency-bound) DMA carries only 32 KB.
        HR = gpool.tile([128, NS * T * SB], bf16, tag="HR", name="HR")
        for s in range(NS):
            nc.scalar.activation(
                HR[:, s * T * SB : (s + 1) * T * SB], H[s], AF.Relu
            )
        out_sb = gpool.tile([128, OUT * NS * SB], f32, tag="outsb", name="outsb")
        KK = NS * SB
        for o in range(OUT):
            P = work.tile([128, NS * T * SB], bf16, tag=f"P{o}", name=f"P{o}")
            nc.vector.tensor_mul(P, lw2_sb[:, o * T * KK : (o + 1) * T * KK], HR)
            acc = work.tile([128, KK], f32, tag=f"acc{o}", name=f"acc{o}")
            nc.vector.tensor_reduce(
                acc,
                P.rearrange("p (kk t) -> p kk t", t=T),
                axis=mybir.AxisListType.X,
                op=add,
            )
            nc.vector.tensor_scalar_add(
                out_sb[:, o * KK : (o + 1) * KK], acc, float(lin_b[o])
            )
            nc.sync.dma_start(
                out=outd[:, o], in_=out_sb[:, o * KK : (o + 1) * KK]
            )

    _split_multi_waits(nc)
    return nc


def _split_multi_waits(nc):
    """Walrus (this build) rejects instructions with more than one sync-wait
    command. Hoist extra waits onto same-engine NoOps placed just before the
    offending instruction — the engine stream blocks on the NoOps first, so
    semantics are identical."""
    from concourse import mybir

    nid = [0]

    def fresh():
        nid[0] += 1
        return f"I-waitsplit-{nid[0]}"

    for bb in nc.main_func.blocks:
        out = []
        for insn in bb.instructions:
            si = insn.sync_info
            if si is not None and si.on_wait and len(si.on_wait) > 1:
                waits = list(si.on_wait)
                for w in waits[:-1]:
                    nop = mybir.InstNoOp(
                        name=fresh(), engine=insn.engine, ins=[], outs=[]
                    )
                    nop.sync_info = mybir.SyncInfo(on_wait=[w], on_update=[])
                    out.append(nop)
                insn.sync_info = mybir.SyncInfo(
                    on_wait=[waits[-1]], on_update=list(si.on_update or [])
                )
            out.append(insn)
        bb.instructions = out


def _host_prep(x, w_ih, lin_w):
    """Build per-core bf16 x shards and the shared constant arrays."""
    import ml_dtypes

    bf = ml_dtypes.bfloat16
    # t-major feature rows: row r = t*FE + f
    xflat = np.ascontiguousarray(
        x.reshape(M, FE, T).transpose(0, 2, 1).reshape(M, C)
    ).astype(bf)
    # column c = b*128 + p holds sequence m = p*(MC/128) + b
    cc = np.arange(MC)
    perm = (cc % 128) * NBLK + cc // 128

    # w3[j, k, tgl] = w_ih[tgl%3, k%32] when tgl//3 == k//32 else 0
    w3 = np.zeros((3, 128, 12), dtype=np.float32)
    for tl in range(4):
        w3[:, tl * 32 : (tl + 1) * 32, tl * 3 : (tl + 1) * 3] = w_ih.T[None]
    w3 = w3.astype(bf)

    # lwb covers both streams, k-major: col o*T*NBLK + kk*T + t = lin_w[o, t]
    import ml_dtypes as _md
    lwb = np.empty((128, OUT * T * NBLK), dtype=_md.bfloat16)
    lwb[:] = np.concatenate(
        [np.tile(lin_w[o].astype(np.float32), NBLK) for o in range(OUT)]
    ).astype(_md.bfloat16)[None, :]

    xts = []
    for c in range(NCORES):
        xc = xflat[c * MC : (c + 1) * MC]
        xt = np.ascontiguousarray(xc[perm].T)          # [C, MC]
        xts.append(
            np.ascontiguousarray(
                xt.reshape(3, 128, NS, 8192 // NS).transpose(0, 2, 1, 3)
            )
        )
    return xts, w3, lwb


def kernel(x, w_ih, w_hh, b_ih, b_hh, lin_w, lin_b, unused=None, **_):
    global _COMPILED
    from concourse.bass_utils import run_bass_kernel_spmd

    x = np.asarray(x, dtype=np.float32)
    w_ih = np.asarray(w_ih, dtype=np.float32)
    w_hh = np.asarray(w_hh, dtype=np.float32).reshape(-1)
    b_ih = np.asarray(b_ih, dtype=np.float32)
    b_hh = np.asarray(b_hh, dtype=np.float32)
    lin_w = np.asarray(lin_w, dtype=np.float32)
    lin_b = np.asarray(lin_b, dtype=np.float32)

    key = (w_hh.tobytes(), b_hh.tobytes(), b_ih.tobytes(), lin_b.tobytes())
    if _COMPILED is None or _COMPILED[1] != key:
        _COMPILED = (_build_program(w_hh, b_hh, b_ih, lin_b), key)
    nc = _COMPILED[0]

    xts, w3, lwb = _host_prep(x, w_ih, lin_w)
    in_maps = [
        {"xt": xts[c], "w3": w3, "lwb": lwb} for c in range(NCORES)
    ]
    res = run_bass_kernel_spmd(nc, in_maps, list(range(NCORES)))
    # device out is [128, OUT, NBLK] per core; row m = p*NBLK + kk
    out = np.concatenate(
        [
            res.results[c]["out"]
            .reshape(128, OUT, NBLK)
            .transpose(0, 2, 1)
            .reshape(MC, OUT)
            for c in range(NCORES)
        ],
        axis=0,
    )
    return np.ascontiguousarray(out.reshape(B, N, OUT))


# revision 30
# speedup vs baseline: 1.0874x; 1.0874x over previous
"""Trainium2 Bass kernel for a hidden-size-1 GRU over M=65536 independent
sequences (T=12 steps, FE=32 features), followed by relu + linear head.

Strategy (data-parallel over 8 NeuronCores, 8192 sequences each):
  - x is cast to bf16 on the host and shipped as [3 chunks, 2 halves,
    128, 4096] so each DMA is one contiguous 1 MiB read; this halves HBM
    traffic (6.3 MiB/core) and the matmul runs at bf16 rate.
  - Gate projection with x as the STATIONARY operand: per 128-sequence
    block, lhsT = x chunk [128 rows = 4 timesteps x 32 features, 128 seqs],
    rhs = a block-diagonal bf16 weight [128, 12], so gates land directly
    as [128 seqs, 12 (t,gate)] tiles in PSUM -- no PE transposes and no
    narrow PSUM copies (the two dominant costs of the previous version).
  - Gate biases are folded into the recurrence's ScalarE activation
    immediates, so the PSUM->SBUF copy is a plain dense [128, 384] copy.
  - The GRU recurrence runs batched in 2 streams of 32 blocks (aligned
    with DMA halves) to hide cross-engine latency; per step it uses the
    form h' = n + z*(h-n) with ops split across Scalar/Vector/GpSimd.
  - relu + linear head: broadcast-multiply with a precomputed lin_w tile
    and a strided tensor_reduce over T, split across Vector and GpSimd.
"""

import numpy as np

B, N, FE, T, OUT = 32, 2048, 32, 12, 3
M = B * N
NCORES = 8
MC = M // NCORES          # 8192 sequences per core
C = FE * T                # 384 contraction length
NBLK = MC // 128          # 64 column blocks of 128 sequences
NS = 2                    # recurrence streams (= DMA halves per chunk)
SB = NBLK // NS           # 32 blocks per stream
HW = MC // NS // 128 * 128 * NS // NS // 1  # unused; kept simple below

_COMPILED = None          # (nc, weights_key)


def _build_program(w_hh, b_hh, b_ih, lin_b):
    """Build the bass program. Recurrent weights/biases are baked as
    immediates (the kernel is JIT-compiled per weight values)."""
    from contextlib import ExitStack

    import concourse.bass as bass
    import concourse.tile as tile
    from concourse import mybir

    f32 = mybir.dt.float32
    bf16 = mybir.dt.bfloat16
    AF = mybir.ActivationFunctionType
    add, mult, sub = (
        mybir.AluOpType.add,
        mybir.AluOpType.mult,
        mybir.AluOpType.subtract,
    )
    wh0, wh1, wh2 = (float(w_hh[i]) for i in range(3))
    br = float(b_ih[0] + b_hh[0])
    bz = float(b_ih[1] + b_hh[1])
    bn = float(b_ih[2])
    bhh2 = float(b_hh[2])

    nc = bass.Bass("TRN2", target_bir_lowering=False, debug=False)

    xd = nc.dram_tensor("xt", [3, NS, 128, 8192 // NS], bf16, kind="ExternalInput").ap()
    w3d = nc.dram_tensor("w3", [3, 128, 12], bf16, kind="ExternalInput").ap()
    lwd = nc.dram_tensor("lwb", [128, OUT * T * SB * NS], bf16, kind="ExternalInput").ap()
    outd = nc.dram_tensor("out", [128, OUT, NS * SB], f32, kind="ExternalOutput").ap()

    with ExitStack() as ctx:
        tc = ctx.enter_context(tile.TileContext(nc))
        consts = ctx.enter_context(tc.tile_pool(name="consts", bufs=1))
        xpool = ctx.enter_context(tc.tile_pool(name="x", bufs=1))
        gpool = ctx.enter_context(tc.tile_pool(name="g", bufs=1))
        work = ctx.enter_context(tc.tile_pool(name="work", bufs=2))
        psum_gp = ctx.enter_context(tc.tile_pool(name="pgp", bufs=4, space="PSUM"))

        # --- constants ---
        w3_sb = consts.tile([128, 36], bf16, tag="w3sb")
        # bias row for the PSUM bias matmul: col b*12 + tl*3 + g -> bias_g
        ones_sb = consts.tile([1, 128], bf16, tag="ones")
        nc.vector.memset(ones_sb, 1.0)
        brow_sb = consts.tile([1, SB * 12], bf16, tag="brow")
        for g, val in enumerate([br, bz, bn]):
            nc.vector.memset(brow_sb.rearrange("p (k g) -> p k g", g=3)[:, :, g], val)

        # --- x stream: 6 contiguous 1 MiB DMAs, chunk-major.  The first x
        # DMA goes out before the small w3/lwb constants so chunk 0 lands as
        # early as possible (the queue drains FIFO).
        xs = {}
        for j in range(3):
            for s in range(NS):
                xs[(j, s)] = xpool.tile(
                    [128, 8192 // NS], bf16, tag=f"x{j}_{s}", name=f"x{j}_{s}"
                )
        nc.sync.dma_start(out=xs[(0, 0)], in_=xd[0, 0])
        for j in range(3):
            nc.sync.dma_start(out=w3_sb[:, j * 12 : (j + 1) * 12], in_=w3d[j])
        for j in range(3):
            for s in range(NS):
                if (j, s) != (0, 0):
                    nc.sync.dma_start(out=xs[(j, s)], in_=xd[j, s])

        # lin_w broadcast tile, loaded after x (needed only for the head)
        lw2_sb = consts.tile([128, OUT * T * SB * NS], bf16, tag="lwsb")
        nc.sync.dma_start(out=lw2_sb, in_=lwd)

        # --- gates: per (chunk, stream): 32 x (ldweights + matmul N=12),
        # then one dense [128, 384] PSUM->SBUF copy into G[s] ---
        # G[s] column layout: (t*3+gate)*SB + k  (tg-major, block-minor)
        G = {}
        for s in range(NS):
            G[s] = gpool.tile([128, 36 * SB], f32, tag=f"G{s}", name=f"G{s}")
        for j in range(3):
            for s in range(NS):
                gp = psum_gp.tile([128, SB * 12], f32, tag="gp", name=f"gp{j}_{s}")
                nc.tensor.matmul(
                    gp,
                    lhsT=ones_sb,
                    rhs=brow_sb,
                    start=True,
                    stop=False,
                    skip_group_check=True,
                )
                for b in range(SB):
                    nc.tensor.matmul(
                        gp[:, b * 12 : (b + 1) * 12],
                        lhsT=xs[(j, s)][:, b * 128 : (b + 1) * 128],
                        rhs=w3_sb[:, j * 12 : (j + 1) * 12],
                        start=False,
                        stop=(b == SB - 1),
                        skip_group_check=True,
                    )
                G3 = G[s].rearrange("p (tg k) -> p tg k", tg=36)
                nc.scalar.activation(
                    G3[:, j * 12 : (j + 1) * 12, :],
                    gp.rearrange("p (k tg) -> p tg k", k=SB),
                    AF.Copy,
                    bias=0.0,
                )

        # --- GRU recurrence, 2 streams of [128, 32] tiles ---
        # h' = n + z*(h - n).  Gate biases already live in G (PSUM bias
        # matmul).  Per stream, r and z share one [128, 64] sigmoid.
        # Emission is software-pipelined (independent ops queued before
        # dependent ones) and DVE work is balanced: Vector runs the cheap
        # STT forms, GpSimd runs u/v and stream-0's tail as tensor_tensor.
        # H is stored k-major (col = k*T + t) so the head is contiguous.
        one = 1.0
        H = {}
        Hv = {}
        for s in range(NS):
            H[s] = gpool.tile([128, T * SB], f32, tag=f"H{s}", name=f"H{s}")
            Hv[s] = H[s].rearrange("p (k t) -> p t k", t=T)

        def gcol(s, t, gate):
            return G[s][:, (t * 3 + gate) * SB : (t * 3 + gate + 1) * SB]

        def wtile(tag, t, s, width=SB):
            return work.tile([128, width], f32, tag=tag, name=f"{tag}_{t}_{s}")

        for t in range(T):
            rz, v, w, n, e, ze = ({} for _ in range(6))
            ht = {s: Hv[s][:, t, :] for s in range(NS)}
            hp = {s: Hv[s][:, t - 1, :] for s in range(NS)} if t else None
            for s in range(NS):
                rz[s] = wtile("rz", t, s, 2 * SB)
                for tag, d in (("v", v), ("w", w), ("n", n), ("e", e), ("ze", ze)):
                    d[s] = wtile(tag, t, s)
            if t == 0:
                for s in range(NS):
                    nc.scalar.activation(rz[s], G[s][:, 0 : 2 * SB], AF.Sigmoid)
                for s in range(NS):
                    nc.vector.tensor_scalar_mul(v[s], rz[s][:, :SB], bhh2)
                    nc.vector.scalar_tensor_tensor(
                        w[s], v[s], one, gcol(s, 0, 2), mult, add
                    )
                for s in range(NS):
                    nc.scalar.activation(n[s], w[s], AF.Tanh)
                nc.gpsimd.tensor_scalar_mul(e[0], n[0], -1.0)
                nc.vector.tensor_scalar_mul(e[1], n[1], -1.0)
            else:
                srz = {s: wtile("srz", t, s, 2 * SB) for s in range(NS)}
                u = {s: wtile("u", t, s) for s in range(NS)}
                for s in range(NS):
                    nc.vector.scalar_tensor_tensor(
                        srz[s][:, :SB], hp[s], wh0, gcol(s, t, 0), mult, add
                    )
                    nc.vector.scalar_tensor_tensor(
                        srz[s][:, SB:], hp[s], wh1, gcol(s, t, 1), mult, add
                    )
                for s in range(NS):
                    nc.scalar.activation(rz[s], srz[s], AF.Sigmoid)
                for s in range(NS):
                    nc.gpsimd.tensor_scalar(u[s], hp[s], wh2, bhh2, mult, add)
                for s in range(NS):
                    nc.vector.scalar_tensor_tensor(
                        v[s], rz[s][:, :SB], one, u[s], mult, mult
                    )
                    nc.vector.scalar_tensor_tensor(
                        w[s], v[s], one, gcol(s, t, 2), mult, add
                    )
                for s in range(NS):
                    nc.scalar.activation(n[s], w[s], AF.Tanh)
                nc.gpsimd.tensor_tensor(e[0], hp[0], n[0], sub)
                nc.vector.scalar_tensor_tensor(e[1], hp[1], one, n[1], mult, sub)
            nc.gpsimd.tensor_mul(ze[0], rz[0][:, SB:], e[0])
            nc.gpsimd.tensor_add(ht[0], n[0], ze[0])
            nc.vector.scalar_tensor_tensor(ze[1], rz[1][:, SB:], one, e[1], mult, mult)
            nc.vector.scalar_tensor_tensor(ht[1], n[1], one, ze[1], mult, add)

        # --- relu + linear head + output DMA, merged across streams ---
        # HR_AB holds both streams' relu(H) side by side (col = (s*SB+k)*T+t,
        # k-major within stream), so each output channel is ONE GpSimd
        # multiply + ONE wide Vector reduce.  Output DMAs go per channel so
        # the last (receipt-latency-bound) DMA carries only 32 KB.
        HR = gpool.tile([128, NS * T * SB], bf16, tag="HR", name="HR")
        for s in range(NS):
            nc.scalar.activation(
                HR[:, s * T * SB : (s + 1) * T * SB], H[s], AF.Relu
            )
        out_sb = gpool.tile([128, OUT * NS * SB], f32, tag="outsb", name="outsb")
        KK = NS * SB
        for o in range(OUT):
            P = work.tile([128, NS * T * SB], bf16, tag=f"P{o}", name=f"P{o}")
            nc.vector.tensor_mul(P, lw2_sb[:, o * T * KK : (o + 1) * T * KK], HR)
            acc = work.tile([128, KK], f32, tag=f"acc{o}", name=f"acc{o}")
            nc.vector.tensor_reduce(
                acc,
                P.rearrange("p (kk t) -> p kk t", t=T),
                axis=mybir.AxisListType.X,
                op=add,
            )
            nc.vector.tensor_scalar_add(
                out_sb[:, o * KK : (o + 1) * KK], acc, float(lin_b[o])
            )
            nc.sync.dma_start(
                out=outd[:, o], in_=out_sb[:, o * KK : (o + 1) * KK]
            )

    _split_multi_waits(nc)
    return nc


def _split_multi_waits(nc):
    """Walrus (this build) rejects instructions with more than one sync-wait
    command. Hoist extra waits onto same-engine NoOps placed just before the
    offending instruction — the engine stream blocks on the NoOps first, so
    semantics are identical."""
    from concourse import mybir

    nid = [0]

    def fresh():
        nid[0] += 1
        return f"I-waitsplit-{nid[0]}"

    for bb in nc.main_func.blocks:
        out = []
        for insn in bb.instructions:
            si = insn.sync_info
            if si is not None and si.on_wait and len(si.on_wait) > 1:
                waits = list(si.on_wait)
                for w in waits[:-1]:
                    nop = mybir.InstNoOp(
                        name=fresh(), engine=insn.engine, ins=[], outs=[]
                    )
                    nop.sync_info = mybir.SyncInfo(on_wait=[w], on_update=[])
                    out.append(nop)
                insn.sync_info = mybir.SyncInfo(
                    on_wait=[waits[-1]], on_update=list(si.on_update or [])
                )
            out.append(insn)
        bb.instructions = out


def _host_prep(x, w_ih, lin_w):
    """Build per-core bf16 x shards and the shared constant arrays."""
    import ml_dtypes

    bf = ml_dtypes.bfloat16
    # t-major feature rows: row r = t*FE + f
    xflat = np.ascontiguousarray(
        x.reshape(M, FE, T).transpose(0, 2, 1).reshape(M, C)
    ).astype(bf)
    # column c = b*128 + p holds sequence m = p*(MC/128) + b
    cc = np.arange(MC)
    perm = (cc % 128) * NBLK + cc // 128

    # w3[j, k, tgl] = w_ih[tgl%3, k%32] when tgl//3 == k//32 else 0
    w3 = np.zeros((3, 128, 12), dtype=np.float32)
    for tl in range(4):
        w3[:, tl * 32 : (tl + 1) * 32, tl * 3 : (tl + 1) * 3] = w_ih.T[None]
    w3 = w3.astype(bf)

    # lwb covers both streams, k-major: col o*T*NBLK + kk*T + t = lin_w[o, t]
    import ml_dtypes as _md
    lwb = np.empty((128, OUT * T * NBLK), dtype=_md.bfloat16)
    lwb[:] = np.concatenate(
        [np.tile(lin_w[o].astype(np.float32), NBLK) for o in range(OUT)]
    ).astype(_md.bfloat16)[None, :]

    xts = []
    for c in range(NCORES):
        xc = xflat[c * MC : (c + 1) * MC]
        xt = np.ascontiguousarray(xc[perm].T)          # [C, MC]
        xts.append(
            np.ascontiguousarray(
                xt.reshape(3, 128, NS, 8192 // NS).transpose(0, 2, 1, 3)
            )
        )
    return xts, w3, lwb


def kernel(x, w_ih, w_hh, b_ih, b_hh, lin_w, lin_b, unused=None, **_):
    global _COMPILED
    from concourse.bass_utils import run_bass_kernel_spmd

    x = np.asarray(x, dtype=np.float32)
    w_ih = np.asarray(w_ih, dtype=np.float32)
    w_hh = np.asarray(w_hh, dtype=np.float32).reshape(-1)
    b_ih = np.asarray(b_ih, dtype=np.float32)
    b_hh = np.asarray(b_hh, dtype=np.float32)
    lin_w = np.asarray(lin_w, dtype=np.float32)
    lin_b = np.asarray(lin_b, dtype=np.float32)

    key = (w_hh.tobytes(), b_hh.tobytes(), b_ih.tobytes(), lin_b.tobytes())
    if _COMPILED is None or _COMPILED[1] != key:
        _COMPILED = (_build_program(w_hh, b_hh, b_ih, lin_b), key)
    nc = _COMPILED[0]

    xts, w3, lwb = _host_prep(x, w_ih, lin_w)
    in_maps = [
        {"xt": xts[c], "w3": w3, "lwb": lwb} for c in range(NCORES)
    ]
    res = run_bass_kernel_spmd(nc, in_maps, list(range(NCORES)))
    # device out is [128, OUT, NBLK] per core; row m = p*NBLK + kk
    out = np.concatenate(
        [
            res.results[c]["out"]
            .reshape(128, OUT, NBLK)
            .transpose(0, 2, 1)
            .reshape(MC, OUT)
            for c in range(NCORES)
        ],
        axis=0,
    )
    return np.ascontiguousarray(out.reshape(B, N, OUT))
